# revision 4
# baseline (speedup 1.0000x reference)
"""AutoCorrelation kernel for Trainium2 (8 NeuronCores, SPMD data-parallel over batch).

Math (derived from the reference nn.Module):
  - R = irfft(rfft(Q) * conj(rfft(K))) is a circular cross-correlation; the
    reference reduces it with mean over (heads, ALL lags).  Sum over all lags
    of a circular cross-correlation factorizes:  sum_tau R[tau] =
    (sum_t Q[t]) * (sum_s K[s]).  So the FFT is algebraically unnecessary --
    only column sums of Q and K are needed, and those are linear in the
    column sums of q and k (sum_t(q @ Wq + bq) = (sum_t q) @ Wq + L*bq).
  - The top-k "delays" are channel indices in [0, 64).  The delay aggregation
    sum_i w_i * roll(V, -d_i) commutes with the output projection AND with the
    value projection, so:  out[t] = sum_d coef_d * U[(t+d) % L]  where
    U = v @ (Wv @ Wo), plus bias (bv @ Wo + bo).  The tap sum is a 64-band
    Toeplitz matmul on the tensor engine.

Device work:
  phase 1: column sums of q[b], k[b] per core via ones-vector matmuls
           (memory bound; bf16 inputs, fp32 PSUM accumulation)
  phase 2: U = v @ W2 per 128-row tile, then out_i = band1^T U_i +
           band2^T U_{i+1} (circular), + bias  (bf16 matmuls, fp32 PSUM)
Host work: [8,512]@[512,512] glue matmuls, top-41 of 64, softmax, band build.
"""

import sys

sys.path.insert(0, "/opt/trn_rl_repo")

import numpy as np

import concourse.bass as bass
import concourse.bacc as bacc
import concourse.mybir as mybir
import concourse.tile as tile
from concourse.bass_utils import run_bass_kernel_spmd

B, L, D, H = 8, 4096, 512, 8
DK = D // H          # 64
K_TOP = 41           # min(int(5*log(4096)), 64)
NCORES = 8
F32 = mybir.dt.float32
BF16 = mybir.dt.bfloat16
NP_BF16 = mybir.dt.np(BF16)

# set by test.py to collect HW profiles
PROFILE = False
TRACE_DIR = None
LAST_HW_TIME_NS = {"phase1": None, "phase2": None}

_NC_CACHE = {}


def _make_nc():
    return bacc.Bacc(
        "TRN2", target_bir_lowering=False, debug=False, num_devices=NCORES
    )


def _build_phase1():
    """Per-core: sums[0, :512] = sum_t q[t, :], sums[0, 512:] = sum_t k[t, :].

    q/k arrive as bf16; sums accumulate in fp32 PSUM via ones-vector matmuls.
    DMA layout: partition p reads rows 8p..8p+7 of its row-group -- an 8 KB
    contiguous chunk per partition (column sums are row-order invariant).
    """
    nc = _make_nc()
    q = nc.dram_tensor("q", [L, D], BF16, kind="ExternalInput")
    k = nc.dram_tensor("k", [L, D], BF16, kind="ExternalInput")
    sums = nc.dram_tensor("sums", [1, 2 * D], F32, kind="ExternalOutput")

    NSUB = 8
    NBIG = L // (128 * NSUB)  # 4

    with tile.TileContext(nc) as tc:
        with (
            tc.tile_pool(name="singles", bufs=1) as singles,
            tc.tile_pool(name="qk", bufs=4) as qk_pool,
            tc.tile_pool(name="ps", bufs=2, space=bass.MemorySpace.PSUM) as ps_pool,
        ):
            ones = singles.tile([128, 1], BF16)
            nc.any.memset(ones[:], 1.0)

            q_re = q.ap().rearrange("(g p n) d -> g p n d", p=128, n=NSUB)
            k_re = k.ap().rearrange("(g p n) d -> g p n d", p=128, n=NSUB)

            psq = ps_pool.tile([1, D], F32)
            psk = ps_pool.tile([1, D], F32)
            for g in range(NBIG):
                tq = qk_pool.tile([128, NSUB, D], BF16, tag="ldq")
                nc.sync.dma_start(tq[:], q_re[g])
                tk = qk_pool.tile([128, NSUB, D], BF16, tag="ldk")
                nc.sync.dma_start(tk[:], k_re[g])
                for c in range(NSUB):
                    nc.tensor.matmul(
                        psq[:1, :],
                        ones[:],
                        tq[:, c, :],
                        start=(g == 0 and c == 0),
                        stop=(g == NBIG - 1 and c == NSUB - 1),
                    )
                for c in range(NSUB):
                    nc.tensor.matmul(
                        psk[:1, :],
                        ones[:],
                        tk[:, c, :],
                        start=(g == 0 and c == 0),
                        stop=(g == NBIG - 1 and c == NSUB - 1),
                    )

            out_sb = singles.tile([1, 2 * D], F32)
            nc.vector.tensor_copy(out_sb[:1, 0:D], psq[:1, :])
            nc.vector.tensor_copy(out_sb[:1, D : 2 * D], psk[:1, :])
            nc.sync.dma_start(sums[:], out_sb[:])

    nc.compile()
    return nc


def _build_phase2():
    """Per-core: out[128i + t, n] = sum_s band1[s, t] * U_i[s, n]
                                  + sum_s band2[s, t] * U_{i+1 mod 32}[s, n] + bias
    with U_i = v[128i : 128(i+1), :] @ W2, computed from host-transposed vT.
    """
    nc = _make_nc()
    vT = nc.dram_tensor("vT", [D, L], BF16, kind="ExternalInput")
    bandsd = nc.dram_tensor("bands", [2, 128, 128], BF16, kind="ExternalInput")
    w2d = nc.dram_tensor("w2", [D, D], BF16, kind="ExternalInput")
    biasd = nc.dram_tensor("bias", [128, D], F32, kind="ExternalInput")
    out = nc.dram_tensor("out", [L, D], F32, kind="ExternalOutput")

    NBLK = L // 128          # 32 tiles / output blocks
    OSUB = 4                 # output blocks per store DMA

    with tile.TileContext(nc) as tc:
        with (
            tc.tile_pool(name="singles", bufs=1) as singles,
            tc.tile_pool(name="usb", bufs=4) as u_pool,
            tc.tile_pool(name="op", bufs=2) as opool,
            tc.tile_pool(name="ups", bufs=2, space=bass.MemorySpace.PSUM) as ups_pool,
            tc.tile_pool(name="ops", bufs=2, space=bass.MemorySpace.PSUM) as ops_pool,
        ):
            # vT: one [128, L] bf16 tile per 128-channel group; per-partition
            # rows are 8 KB contiguous in DRAM.
            vt_re = vT.ap().rearrange("(c p) t -> c p t", p=128)
            vts = []
            for cg in range(4):
                t = singles.tile([128, L], BF16, name=f"vt{cg}")
                nc.sync.dma_start(t[:], vt_re[cg])
                vts.append(t)

            w2_sb = singles.tile([128, 4, D], BF16)
            nc.sync.dma_start(w2_sb[:], w2d.ap().rearrange("(c p) n -> p c n", p=128))
            band_sb = singles.tile([128, 2, 128], BF16)
            nc.sync.dma_start(band_sb[:], bandsd.ap().rearrange("b p t -> p b t"))
            bias_sb = singles.tile([128, D], F32)
            nc.sync.dma_start(bias_sb[:], biasd.ap())

            out_re = out.ap().rearrange("(g n p) d -> g p n d", p=128, n=OSUB)

            def u_tile(i):
                ups = ups_pool.tile([128, D], F32, tag="ups", name=f"ups{i}")
                for cg in range(4):
                    nc.tensor.matmul(
                        ups[:],
                        vts[cg][:, i * 128 : (i + 1) * 128],
                        w2_sb[:, cg, :],
                        start=(cg == 0),
                        stop=(cg == 3),
                    )
                usb = u_pool.tile([128, D], BF16, tag="usb", name=f"usb{i}")
                nc.scalar.copy(usb[:], ups[:])  # ACT: fp32 PSUM -> bf16 SBUF
                return usb

            U = {0: u_tile(0)}
            u_first = singles.tile([128, D], BF16)
            nc.vector.tensor_copy(u_first[:], U[0][:])
            U[1] = u_tile(1)

            ot_tiles = {}
            for i in range(NBLK):
                g, n4 = divmod(i, OSUB)
                if g not in ot_tiles:
                    ot_tiles[g] = opool.tile(
                        [128, OSUB, D], F32, tag="out", name=f"ot{g}"
                    )
                if i + 2 < NBLK:
                    U[i + 2] = u_tile(i + 2)
                u_n = U[i + 1] if i < NBLK - 1 else u_first
                ops = ops_pool.tile([128, D], F32, tag="ops", name=f"ops{i}")
                nc.tensor.matmul(
                    ops[:], band_sb[:, 0, :], U[i][:], start=True, stop=False
                )
                nc.tensor.matmul(
                    ops[:], band_sb[:, 1, :], u_n[:], start=False, stop=True
                )
                del U[i]
                ot = ot_tiles[g]
                nc.vector.tensor_add(ot[:, n4, :], ops[:], bias_sb[:])
                if n4 == OSUB - 1:
                    nc.sync.dma_start(out_re[g], ot[:])
                    del ot_tiles[g]

    nc.compile()
    return nc


_RUN_COUNTER = [0]


def _run(nc, in_maps, phase):
    kwargs = {}
    if PROFILE:
        kwargs["trace"] = True
        if TRACE_DIR is not None:
            import os

            _RUN_COUNTER[0] += 1
            d = os.path.join(TRACE_DIR, f"{phase}_{_RUN_COUNTER[0]}")
            os.makedirs(d, exist_ok=True)
            kwargs["tmpdir"] = d
    res = run_bass_kernel_spmd(nc, in_maps, core_ids=list(range(NCORES)), **kwargs)
    LAST_HW_TIME_NS[phase] = res.exec_time_ns
    return res.results


def kernel(q, k, v, Wq, bq, Wk, bk, Wv, bv, Wo, bo):
    q = np.asarray(q, dtype=np.float32)
    k = np.asarray(k, dtype=np.float32)
    v = np.asarray(v, dtype=np.float32)
    Wq, bq, Wk, bk, Wv, bv, Wo, bo = (
        np.asarray(x, dtype=np.float64) for x in (Wq, bq, Wk, bk, Wv, bv, Wo, bo)
    )

    # ---- phase 1: per-batch column sums of q and k (device) ----
    if "p1" not in _NC_CACHE:
        _NC_CACHE["p1"] = _build_phase1()
    q_bf = q.astype(NP_BF16)
    k_bf = k.astype(NP_BF16)
    in_maps = [{"q": q_bf[b], "k": k_bf[b]} for b in range(B)]
    res1 = _run(_NC_CACHE["p1"], in_maps, "phase1")
    sq = np.stack([res1[b]["sums"][0, :D] for b in range(B)]).astype(np.float64)
    sk = np.stack([res1[b]["sums"][0, D:] for b in range(B)]).astype(np.float64)

    # ---- host glue: top-k channel selection + softmax weights ----
    SQ = sq @ Wq + L * bq                       # [B, D]
    SK = sk @ Wk + L * bk
    m = (SQ.reshape(B, H, DK) * SK.reshape(B, H, DK)).sum(axis=1) / (H * L)  # [B, DK]
    mbar = m.mean(axis=0)
    idx = np.argsort(-mbar, kind="stable")[:K_TOP]
    msel = m[:, idx]
    e = np.exp(msel - msel.max(axis=1, keepdims=True))
    w = e / e.sum(axis=1, keepdims=True)        # [B, K_TOP]
    coef = np.zeros((B, DK))
    coef[:, idx] = w

    # Toeplitz bands: out[t] = sum_d coef[d] * U[(t + d) % L]
    s = np.arange(128)[:, None]
    t = np.arange(128)[None, :]
    d1 = s - t
    d2 = s + 128 - t
    m1 = (d1 >= 0) & (d1 < DK)
    m2 = (d2 >= 0) & (d2 < DK)
    bands = np.zeros((B, 2, 128, 128), dtype=np.float64)
    for b in range(B):
        bands[b, 0] = np.where(m1, coef[b][np.clip(d1, 0, DK - 1)], 0.0)
        bands[b, 1] = np.where(m2, coef[b][np.clip(d2, 0, DK - 1)], 0.0)

    W2 = (Wv @ Wo).astype(np.float32)
    bias2 = (bv @ Wo + bo).astype(np.float32)
    bias_rep = np.ascontiguousarray(np.broadcast_to(bias2, (128, D)))
    w2_bf = W2.astype(NP_BF16)
    bands_bf = bands.astype(NP_BF16)
    vT_bf = np.ascontiguousarray(v.transpose(0, 2, 1)).astype(NP_BF16)  # [B, D, L]

    # ---- phase 2: folded projection + tap aggregation (device) ----
    if "p2" not in _NC_CACHE:
        _NC_CACHE["p2"] = _build_phase2()
    in_maps = [
        {
            "vT": vT_bf[b],
            "bands": np.ascontiguousarray(bands_bf[b]),
            "w2": w2_bf,
            "bias": bias_rep,
        }
        for b in range(B)
    ]
    res2 = _run(_NC_CACHE["p2"], in_maps, "phase2")
    return np.stack([res2[b]["out"] for b in range(B)])


# revision 7
# speedup vs baseline: 1.0015x; 1.0015x over previous
"""AutoCorrelation kernel for Trainium2 (8 NeuronCores, SPMD data-parallel over batch).

Math (derived from the reference nn.Module):
  - R = irfft(rfft(Q) * conj(rfft(K))) is a circular cross-correlation; the
    reference reduces it with mean over (heads, ALL lags).  Sum over all lags
    of a circular cross-correlation factorizes:  sum_tau R[tau] =
    (sum_t Q[t]) * (sum_s K[s]).  So the FFT is algebraically unnecessary --
    only column sums of Q and K are needed, and those are linear in the
    column sums of q and k (sum_t(q @ Wq + bq) = (sum_t q) @ Wq + L*bq).
  - The top-k "delays" are channel indices in [0, 64).  The delay aggregation
    sum_i w_i * roll(V, -d_i) commutes with the output projection AND with the
    value projection, so:  out[t] = sum_d coef_d * U[(t+d) % L]  where
    U = v @ (Wv @ Wo), plus bias (bv @ Wo + bo).  The tap sum is a 64-band
    Toeplitz matmul on the tensor engine.

Device work:
  phase 1: column sums of q[b], k[b] per core via ones-vector matmuls
           (memory bound; bf16 inputs, fp32 PSUM accumulation)
  phase 2: U = v @ W2 per 128-row tile, then out_i = band1^T U_i +
           band2^T U_{i+1} (circular), + bias  (bf16 matmuls, fp32 PSUM)
Host work: [8,512]@[512,512] glue matmuls, top-41 of 64, softmax, band build.
"""

import sys

sys.path.insert(0, "/opt/trn_rl_repo")

import numpy as np

import concourse.bass as bass
import concourse.bacc as bacc
import concourse.mybir as mybir
import concourse.tile as tile
from concourse.bass_utils import run_bass_kernel_spmd

B, L, D, H = 8, 4096, 512, 8
DK = D // H          # 64
K_TOP = 41           # min(int(5*log(4096)), 64)
NCORES = 8
F32 = mybir.dt.float32
BF16 = mybir.dt.bfloat16
NP_BF16 = mybir.dt.np(BF16)

# set by test.py to collect HW profiles
PROFILE = False
TRACE_DIR = None
LAST_HW_TIME_NS = {"phase1": None, "phase2": None}

_NC_CACHE = {}


def _make_nc():
    return bacc.Bacc(
        "TRN2", target_bir_lowering=False, debug=False, num_devices=NCORES
    )


def _build_phase1():
    """Per-core: sums[0, :512] = sum_t q[t, :], sums[0, 512:] = sum_t k[t, :].

    q/k arrive as bf16; sums accumulate in fp32 PSUM via ones-vector matmuls.
    DMA layout: partition p reads rows 8p..8p+7 of its row-group -- an 8 KB
    contiguous chunk per partition (column sums are row-order invariant).
    """
    nc = _make_nc()
    q = nc.dram_tensor("q", [L, D], BF16, kind="ExternalInput")
    k = nc.dram_tensor("k", [L, D], BF16, kind="ExternalInput")
    sums = nc.dram_tensor("sums", [1, 2 * D], F32, kind="ExternalOutput")

    NSUB = 16
    NBIG = L // (128 * NSUB)  # 2

    with tile.TileContext(nc) as tc:
        with (
            tc.tile_pool(name="singles", bufs=1) as singles,
            tc.tile_pool(name="qk", bufs=2) as qk_pool,
            tc.tile_pool(name="ps", bufs=2, space=bass.MemorySpace.PSUM) as ps_pool,
        ):
            ones = singles.tile([128, 1], BF16)
            nc.any.memset(ones[:], 1.0)

            q_re = q.ap().rearrange("(g p n) d -> g p n d", p=128, n=NSUB)
            k_re = k.ap().rearrange("(g p n) d -> g p n d", p=128, n=NSUB)

            psq = ps_pool.tile([1, D], F32)
            psk = ps_pool.tile([1, D], F32)
            for g in range(NBIG):
                tq = qk_pool.tile([128, NSUB, D], BF16, tag="ldq")
                nc.sync.dma_start(tq[:], q_re[g])
                tk = qk_pool.tile([128, NSUB, D], BF16, tag="ldk")
                nc.scalar.dma_start(tk[:], k_re[g])
                for c in range(NSUB):
                    nc.tensor.matmul(
                        psq[:1, :],
                        ones[:],
                        tq[:, c, :],
                        start=(g == 0 and c == 0),
                        stop=(g == NBIG - 1 and c == NSUB - 1),
                    )
                for c in range(NSUB):
                    nc.tensor.matmul(
                        psk[:1, :],
                        ones[:],
                        tk[:, c, :],
                        start=(g == 0 and c == 0),
                        stop=(g == NBIG - 1 and c == NSUB - 1),
                    )

            out_sb = singles.tile([1, 2 * D], F32)
            nc.vector.tensor_copy(out_sb[:1, 0:D], psq[:1, :])
            nc.vector.tensor_copy(out_sb[:1, D : 2 * D], psk[:1, :])
            nc.sync.dma_start(sums[:], out_sb[:])

    nc.compile()
    return nc


def _build_phase2():
    """Per-core: out[128i + t, n] = sum_s band1[s, t] * U_i[s, n]
                                  + sum_s band2[s, t] * U_{i+1 mod 32}[s, n] + bias
    with U_i = v[128i : 128(i+1), :] @ W2, computed from host-transposed vT.
    """
    nc = _make_nc()
    vT = nc.dram_tensor("vT", [D, L], BF16, kind="ExternalInput")
    bandsd = nc.dram_tensor("bands", [2, 128, 128], BF16, kind="ExternalInput")
    w2d = nc.dram_tensor("w2", [D, D], BF16, kind="ExternalInput")
    biasd = nc.dram_tensor("bias", [128, D], F32, kind="ExternalInput")
    out = nc.dram_tensor("out", [L, D], F32, kind="ExternalOutput")

    NBLK = L // 128          # 32 tiles / output blocks
    OSUB = 4                 # output blocks per store DMA

    with tile.TileContext(nc) as tc:
        with (
            tc.tile_pool(name="singles", bufs=1) as singles,
            tc.tile_pool(name="usb", bufs=4) as u_pool,
            tc.tile_pool(name="op", bufs=2) as opool,
            tc.tile_pool(name="ups", bufs=2, space=bass.MemorySpace.PSUM) as ups_pool,
            tc.tile_pool(name="ops", bufs=2, space=bass.MemorySpace.PSUM) as ops_pool,
        ):
            # small constants first so the first U matmul only waits on w2 + vT[0]
            w2_sb = singles.tile([128, 4, D], BF16)
            nc.sync.dma_start(w2_sb[:], w2d.ap().rearrange("(c p) n -> p c n", p=128))
            band_sb = singles.tile([128, 2, 128], BF16)
            nc.scalar.dma_start(band_sb[:], bandsd.ap().rearrange("b p t -> p b t"))
            bias_sb = singles.tile([128, D], F32)
            nc.scalar.dma_start(bias_sb[:], biasd.ap())

            # vT: one [128, L] bf16 tile per 128-channel group; per-partition
            # rows are 8 KB contiguous in DRAM.  Alternate HWDGE rings.
            vt_re = vT.ap().rearrange("(c p) t -> c p t", p=128)
            vts = []
            for cg in range(4):
                t = singles.tile([128, L], BF16, name=f"vt{cg}")
                (nc.sync if cg % 2 == 0 else nc.scalar).dma_start(t[:], vt_re[cg])
                vts.append(t)

            out_re = out.ap().rearrange("(g n p) d -> g p n d", p=128, n=OSUB)

            def u_tile(i):
                ups = ups_pool.tile([128, D], F32, tag="ups", name=f"ups{i}")
                for cg in range(4):
                    nc.tensor.matmul(
                        ups[:],
                        vts[cg][:, i * 128 : (i + 1) * 128],
                        w2_sb[:, cg, :],
                        start=(cg == 0),
                        stop=(cg == 3),
                    )
                usb = u_pool.tile([128, D], BF16, tag="usb", name=f"usb{i}")
                nc.scalar.copy(usb[:], ups[:])  # ACT: fp32 PSUM -> bf16 SBUF
                return usb

            U = {0: u_tile(0)}
            u_first = singles.tile([128, D], BF16)
            nc.vector.tensor_copy(u_first[:], U[0][:])
            U[1] = u_tile(1)

            ot_tiles = {}
            for i in range(NBLK):
                g, n4 = divmod(i, OSUB)
                if g not in ot_tiles:
                    ot_tiles[g] = opool.tile(
                        [128, OSUB, D], F32, tag="out", name=f"ot{g}"
                    )
                if i + 2 < NBLK:
                    U[i + 2] = u_tile(i + 2)
                u_n = U[i + 1] if i < NBLK - 1 else u_first
                ops = ops_pool.tile([128, D], F32, tag="ops", name=f"ops{i}")
                nc.tensor.matmul(
                    ops[:], band_sb[:, 0, :], U[i][:], start=True, stop=False
                )
                nc.tensor.matmul(
                    ops[:], band_sb[:, 1, :], u_n[:], start=False, stop=True
                )
                del U[i]
                ot = ot_tiles[g]
                nc.vector.tensor_add(ot[:, n4, :], ops[:], bias_sb[:])
                if g == (NBLK // OSUB) - 1:
                    # tail: store per-block so the last DMA is small
                    nc.sync.dma_start(out_re[g][:, n4, :], ot[:, n4, :])
                elif n4 == OSUB - 1:
                    nc.sync.dma_start(out_re[g], ot[:])
                    del ot_tiles[g]

    nc.compile()
    return nc


_RUN_COUNTER = [0]


def _run(nc, in_maps, phase):
    kwargs = {}
    if PROFILE:
        kwargs["trace"] = True
        if TRACE_DIR is not None:
            import os

            _RUN_COUNTER[0] += 1
            d = os.path.join(TRACE_DIR, f"{phase}_{_RUN_COUNTER[0]}")
            os.makedirs(d, exist_ok=True)
            kwargs["tmpdir"] = d
    res = run_bass_kernel_spmd(nc, in_maps, core_ids=list(range(NCORES)), **kwargs)
    LAST_HW_TIME_NS[phase] = res.exec_time_ns
    return res.results


def kernel(q, k, v, Wq, bq, Wk, bk, Wv, bv, Wo, bo):
    q = np.asarray(q, dtype=np.float32)
    k = np.asarray(k, dtype=np.float32)
    v = np.asarray(v, dtype=np.float32)
    Wq, bq, Wk, bk, Wv, bv, Wo, bo = (
        np.asarray(x, dtype=np.float64) for x in (Wq, bq, Wk, bk, Wv, bv, Wo, bo)
    )

    # ---- phase 1: per-batch column sums of q and k (device) ----
    if "p1" not in _NC_CACHE:
        _NC_CACHE["p1"] = _build_phase1()
    q_bf = q.astype(NP_BF16)
    k_bf = k.astype(NP_BF16)
    in_maps = [{"q": q_bf[b], "k": k_bf[b]} for b in range(B)]
    res1 = _run(_NC_CACHE["p1"], in_maps, "phase1")
    sq = np.stack([res1[b]["sums"][0, :D] for b in range(B)]).astype(np.float64)
    sk = np.stack([res1[b]["sums"][0, D:] for b in range(B)]).astype(np.float64)

    # ---- host glue: top-k channel selection + softmax weights ----
    SQ = sq @ Wq + L * bq                       # [B, D]
    SK = sk @ Wk + L * bk
    m = (SQ.reshape(B, H, DK) * SK.reshape(B, H, DK)).sum(axis=1) / (H * L)  # [B, DK]
    mbar = m.mean(axis=0)
    idx = np.argsort(-mbar, kind="stable")[:K_TOP]
    msel = m[:, idx]
    e = np.exp(msel - msel.max(axis=1, keepdims=True))
    w = e / e.sum(axis=1, keepdims=True)        # [B, K_TOP]
    coef = np.zeros((B, DK))
    coef[:, idx] = w

    # Toeplitz bands: out[t] = sum_d coef[d] * U[(t + d) % L]
    s = np.arange(128)[:, None]
    t = np.arange(128)[None, :]
    d1 = s - t
    d2 = s + 128 - t
    m1 = (d1 >= 0) & (d1 < DK)
    m2 = (d2 >= 0) & (d2 < DK)
    bands = np.zeros((B, 2, 128, 128), dtype=np.float64)
    for b in range(B):
        bands[b, 0] = np.where(m1, coef[b][np.clip(d1, 0, DK - 1)], 0.0)
        bands[b, 1] = np.where(m2, coef[b][np.clip(d2, 0, DK - 1)], 0.0)

    W2 = (Wv @ Wo).astype(np.float32)
    bias2 = (bv @ Wo + bo).astype(np.float32)
    bias_rep = np.ascontiguousarray(np.broadcast_to(bias2, (128, D)))
    w2_bf = W2.astype(NP_BF16)
    bands_bf = bands.astype(NP_BF16)
    vT_bf = np.ascontiguousarray(v.transpose(0, 2, 1)).astype(NP_BF16)  # [B, D, L]

    # ---- phase 2: folded projection + tap aggregation (device) ----
    if "p2" not in _NC_CACHE:
        _NC_CACHE["p2"] = _build_phase2()
    in_maps = [
        {
            "vT": vT_bf[b],
            "bands": np.ascontiguousarray(bands_bf[b]),
            "w2": w2_bf,
            "bias": bias_rep,
        }
        for b in range(B)
    ]
    res2 = _run(_NC_CACHE["p2"], in_maps, "phase2")
    return np.stack([res2[b]["out"] for b in range(B)])


# revision 11
# speedup vs baseline: 1.0383x; 1.0368x over previous
"""AutoCorrelation kernel for Trainium2 (8 NeuronCores, SPMD data-parallel over batch).

Math (derived from the reference nn.Module):
  - R = irfft(rfft(Q) * conj(rfft(K))) is a circular cross-correlation; the
    reference reduces it with mean over (heads, ALL lags).  Sum over all lags
    of a circular cross-correlation factorizes:  sum_tau R[tau] =
    (sum_t Q[t]) * (sum_s K[s]).  So the FFT is algebraically unnecessary --
    only column sums of Q and K are needed, and those are linear in the
    column sums of q and k (sum_t(q @ Wq + bq) = (sum_t q) @ Wq + L*bq).
  - The top-k "delays" are channel indices in [0, 64).  The delay aggregation
    sum_i w_i * roll(V, -d_i) commutes with the output projection AND with the
    value projection, so:  out[t] = sum_d coef_d * U[(t+d) % L]  where
    U = v @ (Wv @ Wo), plus bias (bv @ Wo + bo).  The tap sum is a 64-band
    Toeplitz matmul on the tensor engine.

Device work:
  phase 1: column sums of q[b], k[b] per core via ones-vector matmuls
           (memory bound; bf16 inputs, fp32 PSUM accumulation)
  phase 2: U = v @ W2 per 128-row tile, then out_i = band1^T U_i +
           band2^T U_{i+1} (circular), + bias  (bf16 matmuls, fp32 PSUM)
Host work: [8,512]@[512,512] glue matmuls, top-41 of 64, softmax, band build.
"""

import sys

sys.path.insert(0, "/opt/trn_rl_repo")

import numpy as np

import concourse.bass as bass
import concourse.bacc as bacc
import concourse.mybir as mybir
import concourse.tile as tile
from concourse.bass_utils import run_bass_kernel_spmd

B, L, D, H = 8, 4096, 512, 8
DK = D // H          # 64
K_TOP = 41           # min(int(5*log(4096)), 64)
NCORES = 8
F32 = mybir.dt.float32
BF16 = mybir.dt.bfloat16
NP_BF16 = mybir.dt.np(BF16)

# set by test.py to collect HW profiles
PROFILE = False
TRACE_DIR = None
LAST_HW_TIME_NS = {"phase1": None, "phase2": None}

_NC_CACHE = {}


def _make_nc():
    return bacc.Bacc(
        "TRN2", target_bir_lowering=False, debug=False, num_devices=NCORES
    )


def _build_phase1():
    """Per-core: sums[0, :512] = sum_t q[t, :], sums[0, 512:] = sum_t k[t, :].

    q/k arrive as bf16; sums accumulate in fp32 PSUM via ones-vector matmuls.
    DMA layout: partition p reads rows 8p..8p+7 of its row-group -- an 8 KB
    contiguous chunk per partition (column sums are row-order invariant).
    """
    nc = _make_nc()
    q = nc.dram_tensor("q", [L, D], BF16, kind="ExternalInput")
    k = nc.dram_tensor("k", [L, D], BF16, kind="ExternalInput")
    sums = nc.dram_tensor("sums", [1, 2 * D], F32, kind="ExternalOutput")

    NSUB = 8
    NBIG = L // (128 * NSUB)  # 4

    with tile.TileContext(nc) as tc:
        with (
            tc.tile_pool(name="singles", bufs=1) as singles,
            tc.tile_pool(name="qk", bufs=2) as qk_pool,
            tc.tile_pool(name="ps", bufs=2, space=bass.MemorySpace.PSUM) as ps_pool,
        ):
            ones = singles.tile([128, 1], BF16)
            nc.any.memset(ones[:], 1.0)

            q_re = q.ap().rearrange("(g p n) d -> g p n d", p=128, n=NSUB)
            k_re = k.ap().rearrange("(g p n) d -> g p n d", p=128, n=NSUB)

            psq = ps_pool.tile([1, D], F32)
            psk = ps_pool.tile([1, D], F32)
            for g in range(NBIG):
                tq = qk_pool.tile([128, NSUB, D], BF16, tag="ldq")
                nc.sync.dma_start(tq[:], q_re[g])
                tk = qk_pool.tile([128, NSUB, D], BF16, tag="ldk")
                nc.scalar.dma_start(tk[:], k_re[g])
                for c in range(NSUB):
                    nc.tensor.matmul(
                        psq[:1, :],
                        ones[:],
                        tq[:, c, :],
                        start=(g == 0 and c == 0),
                        stop=(g == NBIG - 1 and c == NSUB - 1),
                    )
                for c in range(NSUB):
                    nc.tensor.matmul(
                        psk[:1, :],
                        ones[:],
                        tk[:, c, :],
                        start=(g == 0 and c == 0),
                        stop=(g == NBIG - 1 and c == NSUB - 1),
                    )

            out_sb = singles.tile([1, 2 * D], F32)
            nc.vector.tensor_copy(out_sb[:1, 0:D], psq[:1, :])
            nc.vector.tensor_copy(out_sb[:1, D : 2 * D], psk[:1, :])
            nc.sync.dma_start(sums[:], out_sb[:])

    nc.compile()
    return nc


def _build_phase2():
    """Per-core: out[128i + t, n] = sum_s band1[s, t] * U_i[s, n]
                                  + sum_s band2[s, t] * U_{i+1 mod 32}[s, n] + bias
    with U_i = v[128i : 128(i+1), :] @ W2, computed from host-transposed vT.
    """
    nc = _make_nc()
    vT = nc.dram_tensor("vT", [D, L], BF16, kind="ExternalInput")
    bandsd = nc.dram_tensor("bands", [2, 128, 128], BF16, kind="ExternalInput")
    w2d = nc.dram_tensor("w2", [D, D], BF16, kind="ExternalInput")
    biasd = nc.dram_tensor("bias", [128, D], F32, kind="ExternalInput")
    out = nc.dram_tensor("out", [L, D], F32, kind="ExternalOutput")

    NBLK = L // 128          # 32 tiles / output blocks
    OSUB = 4                 # output blocks per store DMA

    with tile.TileContext(nc) as tc:
        with (
            tc.tile_pool(name="singles", bufs=1) as singles,
            tc.tile_pool(name="usb", bufs=6) as u_pool,
            tc.tile_pool(name="op", bufs=2) as opool,
            tc.tile_pool(name="ups", bufs=4, space=bass.MemorySpace.PSUM) as ups_pool,
            tc.tile_pool(name="ops", bufs=2, space=bass.MemorySpace.PSUM) as ops_pool,
        ):
            # vT: one [128, L] bf16 tile per 128-channel group; per-partition
            # rows are 8 KB contiguous in DRAM.  vT[0] leads on the sync ring
            # (first U matmuls need it); constants ride the scalar ring.
            vt_re = vT.ap().rearrange("(c p) t -> c p t", p=128)
            vts = [singles.tile([128, L], BF16, name=f"vt{cg}") for cg in range(4)]
            nc.sync.dma_start(vts[0][:], vt_re[0])
            w2_sb = singles.tile([128, 4, D], BF16)
            nc.scalar.dma_start(
                w2_sb[:], w2d.ap().rearrange("(c p) n -> p c n", p=128)
            )
            nc.sync.dma_start(vts[1][:], vt_re[1])
            nc.sync.dma_start(vts[2][:], vt_re[2])
            nc.scalar.dma_start(vts[3][:], vt_re[3])
            band_sb = singles.tile([128, 2, 128], BF16)
            nc.scalar.dma_start(band_sb[:], bandsd.ap().rearrange("b p t -> p b t"))
            bias_sb = singles.tile([128, D], F32)
            nc.scalar.dma_start(bias_sb[:], biasd.ap())

            out_re = out.ap().rearrange("(g n p) d -> g p n d", p=128, n=OSUB)

            def u_mm(ups, i, cg):
                nc.tensor.matmul(
                    ups[:],
                    vts[cg][:, i * 128 : (i + 1) * 128],
                    w2_sb[:, cg, :],
                    start=(cg == 0),
                    stop=(cg == 3),
                )

            def u_cast(ups, i):
                usb = u_pool.tile([128, D], BF16, tag="usb", name=f"usb{i}")
                nc.scalar.copy(usb[:], ups[:])  # ACT: fp32 PSUM -> bf16 SBUF
                return usb

            def u_tile(i):
                ups = ups_pool.tile([128, D], F32, tag="ups", name=f"ups{i}")
                for cg in range(4):
                    u_mm(ups, i, cg)
                return u_cast(ups, i)

            # Prologue: first NPRO tiles in cg-major order so the PE starts as
            # soon as vT[0] has landed instead of waiting for all of vT.
            NPRO = 4
            U = {}
            pro_ups = [
                ups_pool.tile([128, D], F32, tag="ups", name=f"ups{i}")
                for i in range(NPRO)
            ]
            for cg in range(4):
                for i in range(NPRO):
                    u_mm(pro_ups[i], i, cg)
            for i in range(NPRO):
                U[i] = u_cast(pro_ups[i], i)
            u_first = singles.tile([128, D], BF16)
            nc.vector.tensor_copy(u_first[:], U[0][:])

            ot_tiles = {}
            for i in range(NBLK):
                g, n4 = divmod(i, OSUB)
                if g not in ot_tiles:
                    ot_tiles[g] = opool.tile(
                        [128, OSUB, D], F32, tag="out", name=f"ot{g}"
                    )
                if NPRO <= i + 2 < NBLK:
                    U[i + 2] = u_tile(i + 2)
                u_n = U[i + 1] if i < NBLK - 1 else u_first
                ops = ops_pool.tile([128, D], F32, tag="ops", name=f"ops{i}")
                nc.tensor.matmul(
                    ops[:], band_sb[:, 0, :], U[i][:], start=True, stop=False
                )
                nc.tensor.matmul(
                    ops[:], band_sb[:, 1, :], u_n[:], start=False, stop=True
                )
                del U[i]
                ot = ot_tiles[g]
                nc.vector.tensor_add(ot[:, n4, :], ops[:], bias_sb[:])
                if g == (NBLK // OSUB) - 1:
                    # tail: store per-block so the last DMA is small
                    nc.sync.dma_start(out_re[g][:, n4, :], ot[:, n4, :])
                elif n4 == OSUB - 1:
                    nc.sync.dma_start(out_re[g], ot[:])
                    del ot_tiles[g]

    nc.compile()
    return nc


_RUN_COUNTER = [0]


def _run(nc, in_maps, phase):
    kwargs = {}
    if PROFILE:
        kwargs["trace"] = True
        if TRACE_DIR is not None:
            import os

            _RUN_COUNTER[0] += 1
            d = os.path.join(TRACE_DIR, f"{phase}_{_RUN_COUNTER[0]}")
            os.makedirs(d, exist_ok=True)
            kwargs["tmpdir"] = d
    res = run_bass_kernel_spmd(nc, in_maps, core_ids=list(range(NCORES)), **kwargs)
    LAST_HW_TIME_NS[phase] = res.exec_time_ns
    return res.results


def kernel(q, k, v, Wq, bq, Wk, bk, Wv, bv, Wo, bo):
    q = np.asarray(q, dtype=np.float32)
    k = np.asarray(k, dtype=np.float32)
    v = np.asarray(v, dtype=np.float32)
    Wq, bq, Wk, bk, Wv, bv, Wo, bo = (
        np.asarray(x, dtype=np.float64) for x in (Wq, bq, Wk, bk, Wv, bv, Wo, bo)
    )

    # ---- phase 1: per-batch column sums of q and k (device) ----
    if "p1" not in _NC_CACHE:
        _NC_CACHE["p1"] = _build_phase1()
    q_bf = q.astype(NP_BF16)
    k_bf = k.astype(NP_BF16)
    in_maps = [{"q": q_bf[b], "k": k_bf[b]} for b in range(B)]
    res1 = _run(_NC_CACHE["p1"], in_maps, "phase1")
    sq = np.stack([res1[b]["sums"][0, :D] for b in range(B)]).astype(np.float64)
    sk = np.stack([res1[b]["sums"][0, D:] for b in range(B)]).astype(np.float64)

    # ---- host glue: top-k channel selection + softmax weights ----
    SQ = sq @ Wq + L * bq                       # [B, D]
    SK = sk @ Wk + L * bk
    m = (SQ.reshape(B, H, DK) * SK.reshape(B, H, DK)).sum(axis=1) / (H * L)  # [B, DK]
    mbar = m.mean(axis=0)
    idx = np.argsort(-mbar, kind="stable")[:K_TOP]
    msel = m[:, idx]
    e = np.exp(msel - msel.max(axis=1, keepdims=True))
    w = e / e.sum(axis=1, keepdims=True)        # [B, K_TOP]
    coef = np.zeros((B, DK))
    coef[:, idx] = w

    # Toeplitz bands: out[t] = sum_d coef[d] * U[(t + d) % L]
    s = np.arange(128)[:, None]
    t = np.arange(128)[None, :]
    d1 = s - t
    d2 = s + 128 - t
    m1 = (d1 >= 0) & (d1 < DK)
    m2 = (d2 >= 0) & (d2 < DK)
    bands = np.zeros((B, 2, 128, 128), dtype=np.float64)
    for b in range(B):
        bands[b, 0] = np.where(m1, coef[b][np.clip(d1, 0, DK - 1)], 0.0)
        bands[b, 1] = np.where(m2, coef[b][np.clip(d2, 0, DK - 1)], 0.0)

    W2 = (Wv @ Wo).astype(np.float32)
    bias2 = (bv @ Wo + bo).astype(np.float32)
    bias_rep = np.ascontiguousarray(np.broadcast_to(bias2, (128, D)))
    w2_bf = W2.astype(NP_BF16)
    bands_bf = bands.astype(NP_BF16)
    vT_bf = np.ascontiguousarray(v.transpose(0, 2, 1)).astype(NP_BF16)  # [B, D, L]

    # ---- phase 2: folded projection + tap aggregation (device) ----
    if "p2" not in _NC_CACHE:
        _NC_CACHE["p2"] = _build_phase2()
    in_maps = [
        {
            "vT": vT_bf[b],
            "bands": np.ascontiguousarray(bands_bf[b]),
            "w2": w2_bf,
            "bias": bias_rep,
        }
        for b in range(B)
    ]
    res2 = _run(_NC_CACHE["p2"], in_maps, "phase2")
    return np.stack([res2[b]["out"] for b in range(B)])
